# revision 31
# baseline (speedup 1.0000x reference)
"""Trainium2 Bass kernel for nn_CrossAttention (B=4, LQ=4096, S=4096, D=512).

Sharding: data-parallel over (batch, query-half): core = 2*b + half.
Each core handles one batch element and 2048 query rows.

Algebraic fusion removes the per-pair duplicated K/V projections:
  scores^T = tgt @ A          with A = wk @ q^T   (bk cancels in softmax)
  ctx^T    = wv^T @ U         with U = tgt^T @ probs^T
so the only S-sized matmuls are the two intrinsic attention GEMMs; all
projection-sized work scales with the per-core query count.

Precision: scores path runs in fp16 (3 extra mantissa bits over bf16 at the
same PE rate). Softmax skips max-subtraction but shifts by a global C=18 so
the unnormalized exp fits fp16; the shift cancels exactly in the final
per-row normalization. U / ctx / output projection run bf16.

Scheduling: everything outside the two big attention GEMMs is expressed as
small closures injected one-per-step into the attention loops so the PE
never drains: the target load/cast/transpose stream rides block 0's loop
(staggered one chunk ahead so the ACT copy is hidden), each block's query
projections ride the previous block's loop, and each block's tail (rowsum
reciprocal, ctx and output projections) rides the next block's loop.
Engine queues are ordered by data-arrival time so no in-order queue blocks
on a late DMA.
"""

import numpy as np

B, LQ, S = 4, 4096, 4096
D = 512          # SRC == TGT == 512
P = 128
LQH = LQ // 2    # 2048 query rows per core
DC = D // P      # 4 chunks of the feature dims
SC = S // P      # 32 s-chunks of 128
IB = 512         # query block width
NB = LQH // IB   # 4 query blocks
CEXP = 18.0      # global softmax shift: exp(score - CEXP) fits fp16

_CACHED = {}


def _build_program():
    import concourse.bass as bass
    import concourse.mybir as mybir
    import concourse.tile as tile
    from concourse import bacc
    from contextlib import ExitStack

    f32 = mybir.dt.float32
    bf16 = mybir.dt.bfloat16
    f16 = mybir.dt.float16
    AF = mybir.ActivationFunctionType
    OP = mybir.AluOpType

    nc = bacc.Bacc("TRN2", target_bir_lowering=False, debug=False, num_devices=8)

    query = nc.dram_tensor("query", [LQH, D], f32, kind="ExternalInput").ap()
    target = nc.dram_tensor("target", [S, D], f32, kind="ExternalInput").ap()
    w_dram = {}
    b_dram = {}
    for nm in ("wq", "wk", "wv", "wo"):
        w_dram[nm] = nc.dram_tensor(nm, [D, D], f32, kind="ExternalInput").ap()
    for nm in ("bq", "bk", "bv", "bo"):
        b_dram[nm] = nc.dram_tensor(nm, [D], f32, kind="ExternalInput").ap()
    ident16_d = nc.dram_tensor("ident16", [P, P], f16, kind="ExternalInput").ap()
    identf_d = nc.dram_tensor("identf", [P, P], f32, kind="ExternalInput").ap()
    bias_scr = nc.dram_tensor("bias_scr", [D], f32, kind="Internal").ap()
    out_dram = nc.dram_tensor("out", [LQH, D], f32, kind="ExternalOutput").ap()

    with tile.TileContext(nc) as tc, ExitStack() as ctx:
        const = ctx.enter_context(tc.tile_pool(name="const", bufs=1))
        big = ctx.enter_context(tc.tile_pool(name="big", bufs=1))
        wstage = ctx.enter_context(tc.tile_pool(name="wstage", bufs=2))
        ld = ctx.enter_context(tc.tile_pool(name="ld", bufs=4))
        tcp = ctx.enter_context(tc.tile_pool(name="tcp", bufs=10))
        qp = ctx.enter_context(tc.tile_pool(name="qp", bufs=2))
        ptp = ctx.enter_context(tc.tile_pool(name="ptp", bufs=4))
        up = ctx.enter_context(tc.tile_pool(name="up", bufs=2))
        outp = ctx.enter_context(tc.tile_pool(name="outp", bufs=2))
        smallp = ctx.enter_context(tc.tile_pool(name="smallp", bufs=2))
        # PSUM budget: work 3 banks + U 4 banks + rs1 1 bank = 8
        ps_work = ctx.enter_context(tc.tile_pool(name="ps_work", bufs=3, space="PSUM"))
        ps_u = ctx.enter_context(tc.tile_pool(name="ps_u", bufs=1, space="PSUM"))
        ps_rs1 = ctx.enter_context(tc.tile_pool(name="ps_rs1", bufs=1, space="PSUM"))

        # ---- constants ----
        ident16 = const.tile([P, P], f16, tag="ident16", name="ident16")
        nc.sync.dma_start(ident16, ident16_d)

        def transpose16(out_f32, in_f16):
            # fp16 128x128 transpose as a regular matmul: out = in^T @ I
            nc.tensor.matmul(out_f32, in_f16, ident16, start=True, stop=True)

        ones_col = const.tile([P, 1], f32, tag="ones_col", name="ones_col")
        nc.vector.memset(ones_col, 1.0)
        negc_col = const.tile([P, 1], f32, tag="negc_col", name="negc_col")
        nc.vector.memset(negc_col, -CEXP)
        rstage = const.tile([P, IB], f32, tag="rstage", name="rstage")
        nc.vector.memset(rstage, 0.0)

        bq_col = const.tile([P, DC], f32, tag="bq_col", name="bq_col")
        nc.gpsimd.dma_start(out=bq_col, in_=b_dram["bq"].rearrange("(c p) -> p c", p=P))
        bv_col = const.tile([P, DC], f32, tag="bv_col", name="bv_col")
        nc.gpsimd.dma_start(out=bv_col, in_=b_dram["bv"].rearrange("(c p) -> p c", p=P))
        bo_row = const.tile([1, D], f32, tag="bo_row", name="bo_row")
        nc.gpsimd.dma_start(out=bo_row, in_=b_dram["bo"])

        # ---- persistent SBUF arrays ----
        wq_h = const.tile([P, DC, D], f16, tag="wq_h", name="wq_h")
        wk_h = const.tile([P, DC, D], f16, tag="wk_h", name="wk_h")
        wkT_h = const.tile([P, DC, D], f16, tag="wkT_h", name="wkT_h")
        wv_b = const.tile([P, DC, D], bf16, tag="wv_b", name="wv_b")
        wo_b = const.tile([P, DC, D], bf16, tag="wo_b", name="wo_b")
        A_sb = big.tile([P, DC, LQH], f16, tag="A_sb", name="A_sb")
        tgtT = big.tile([P, DC, S], f16, tag="tgtT", name="tgtT")
        tgt_h = big.tile([P, SC, D], f16, tag="tgt_h", name="tgt_h")
        bias_rep = const.tile([P, D], f32, tag="bias_rep", name="bias_rep")

        # ---- DMA issue, ordered by when the data is needed ----
        # weight loads are split per 128-row piece so consumers can start on
        # the first piece; queue order tracks ring-arrival order
        qstage = {}
        for cc in range(IB // P):
            chunk = ld.tile([P, D], f32, tag="qc", name=f"qc_0_{cc}")
            nc.sync.dma_start(chunk, query[cc * P:(cc + 1) * P, :])
            qstage[(0, cc)] = chunk
        wq_f = wstage.tile([P, DC, D], f32, tag="w_stage", name="wq_f32")
        wk_f = wstage.tile([P, DC, D], f32, tag="w_stage", name="wk_f32")
        for dc in range(DC):
            nc.scalar.dma_start(wq_f[:, dc, :], w_dram["wq"][dc * P:(dc + 1) * P, :])
        for dc in range(DC):
            nc.scalar.dma_start(wk_f[:, dc, :], w_dram["wk"][dc * P:(dc + 1) * P, :])
        identf = const.tile([P, P], f32, tag="identf", name="identf")
        nc.scalar.dma_start(identf, identf_d)
        tstage = []
        for scc in range(8):
            chunk = tcp.tile([P, D], f32, tag="tc", name=f"tc_{scc}")
            nc.sync.dma_start(chunk, target[scc * P:(scc + 1) * P, :])
            tstage.append(chunk)
        for ibk in range(1, NB):
            for cc in range(IB // P):
                chunk = ld.tile([P, D], f32, tag="qc", name=f"qc_{ibk}_{cc}")
                nc.sync.dma_start(chunk, query[ibk * IB + cc * P: ibk * IB + (cc + 1) * P, :])
                qstage[(ibk, cc)] = chunk
        wv_f = wstage.tile([P, DC, D], f32, tag="w_stage", name="wv_f32")
        wo_f = wstage.tile([P, DC, D], f32, tag="w_stage", name="wo_f32")
        for dc in range(DC):
            nc.sync.dma_start(wv_f[:, dc, :], w_dram["wv"][dc * P:(dc + 1) * P, :])
        for dc in range(DC):
            nc.sync.dma_start(wo_f[:, dc, :], w_dram["wo"][dc * P:(dc + 1) * P, :])
        for scc in range(8, SC):
            chunk = tcp.tile([P, D], f32, tag="tc", name=f"tc_{scc}")
            nc.sync.dma_start(chunk, target[scc * P:(scc + 1) * P, :])
            tstage.append(chunk)

        # ---- query-side compute pieces for one ib block ----
        def qside_ops(ibk):
            """Closures: 4 transpose-chunk ops, 4 q-proj ops, 4 A ops."""
            qinT = qp.tile([P, DC, IB], f16, tag="qinT", name=f"qinT{ibk}")
            qT = qp.tile([P, DC, IB], f16, tag="qT", name=f"qT{ibk}")
            ops = []

            for cc in range(IB // P):
                def chunk_op(cc=cc):
                    chunk = qstage[(ibk, cc)]
                    cast = ld.tile([P, D], f16, tag="qch", name=f"qch_{ibk}_{cc}")
                    nc.vector.tensor_copy(out=cast, in_=chunk)
                    psv = ps_work.tile([P, D], f32, tag="work", name=f"pst_q{ibk}_{cc}")
                    for dc in range(DC):
                        transpose16(psv[:, dc * P:(dc + 1) * P],
                                    cast[:, dc * P:(dc + 1) * P])
                    c = cc * P
                    nc.scalar.activation(qinT[:, :, c:c + P],
                                         psv.rearrange("p (c q) -> p c q", c=DC),
                                         AF.Copy)
                ops.append(chunk_op)

            for tcc in range(DC):
                def qproj_op(tcc=tcc):
                    ps = ps_work.tile([P, IB], f32, tag="work", name=f"psq_{tcc}_{ibk}")
                    for dc in range(DC):
                        nc.tensor.matmul(ps, wq_h[:, dc, tcc * P:(tcc + 1) * P],
                                         qinT[:, dc, :],
                                         start=(dc == 0), stop=(dc == DC - 1))
                    nc.vector.tensor_tensor(qT[:, tcc, :], ps,
                                            bq_col[:, tcc:tcc + 1].to_broadcast([P, IB]),
                                            OP.add)
                ops.append(qproj_op)

            for c_tin in range(DC):
                def a_op(c_tin=c_tin):
                    ps = ps_work.tile([P, IB], f32, tag="work", name=f"psA_{c_tin}_{ibk}")
                    for c_tout in range(DC):
                        nc.tensor.matmul(
                            ps,
                            wkT_h[:, c_tout, c_tin * P:(c_tin + 1) * P],
                            qT[:, c_tout, :],
                            start=(c_tout == 0), stop=(c_tout == DC - 1))
                    nc.scalar.activation(A_sb[:, c_tin, ibk * IB:(ibk + 1) * IB],
                                         ps, AF.Copy)
                ops.append(a_op)
            return ops

        # ---- deferred weight casts + output-bias setup (ride block 0) ----
        def wcast_ops():
            ops = []
            for dst, src in ((wv_b, wv_f), (wo_b, wo_f)):
                for dc in range(DC):
                    def cast_op(dst=dst, src=src, dc=dc):
                        nc.vector.tensor_copy(out=dst[:, dc, :], in_=src[:, dc, :])
                    ops.append(cast_op)
            return ops

        def bias_setup_op():
            bv_colb = const.tile([P, DC], bf16, tag="bv_colb", name="bv_colb")
            nc.vector.tensor_copy(out=bv_colb, in_=bv_col)
            bvwo_ps = ps_rs1.tile([1, D], f32, tag="rs1", name="bvwo")
            for c in range(DC):
                nc.tensor.matmul(bvwo_ps, bv_colb[:, c:c + 1], wo_b[:, c, :],
                                 start=(c == 0), stop=(c == DC - 1))
            bias_tot = const.tile([1, D], f32, tag="bias_tot", name="bias_tot")
            nc.vector.tensor_tensor(bias_tot, bvwo_ps, bo_row, OP.add)
            nc.sync.dma_start(out=bias_scr, in_=bias_tot)
            bcast = bass.AP(tensor=bias_scr.tensor, offset=bias_scr.offset,
                            ap=[[0, P]] + list(bias_scr.ap))
            nc.sync.dma_start(out=bias_rep, in_=bcast)

        # ---- per-block tail: rowsum recip, ctx projection, out projection ----
        def make_tail(ib, U_sb, acc):
            ops = []
            rc_sb = smallp.tile([P, DC], f32, tag="rc_sb", name=f"rc_{ib}")
            ctxT = outp.tile([P, DC, IB], bf16, tag="ctx", name=f"ctxT_{ib}")

            def rs_collapse():
                rs_ps = ps_rs1.tile([1, IB], f32, tag="rs1", name=f"rs_{ib}")
                nc.tensor.matmul(rs_ps, ones_col, acc, start=True, stop=True)
                nc.vector.tensor_copy(out=rstage[0:1, :], in_=rs_ps)
            ops.append(rs_collapse)

            def rs_recip():
                rt_ps = ps_work.tile([P, IB], f32, tag="work", name=f"rt_{ib}")
                for ic in range(DC):
                    nc.tensor.transpose(rt_ps[:, ic * P:(ic + 1) * P],
                                        rstage[:, ic * P:(ic + 1) * P], identf)
                rsum_col = smallp.tile([P, DC], f32, tag="rsum_col", name=f"rsc_{ib}")
                nc.scalar.activation(rsum_col,
                                     rt_ps.rearrange("p (c q) -> p c q", c=DC)[:, :, 0],
                                     AF.Copy)
                nc.vector.reciprocal(rc_sb, rsum_col)

            ctx_ops = []
            for c_tout in range(DC):
                def ctx_mm(c_tout=c_tout):
                    ps = ps_work.tile([P, IB], f32, tag="work", name=f"ctx_{ib}_{c_tout}")
                    for c_tin in range(DC):
                        nc.tensor.matmul(ps,
                                         wv_b[:, c_tin, c_tout * P:(c_tout + 1) * P],
                                         U_sb[:, c_tin, :],
                                         start=(c_tin == 0), stop=(c_tin == DC - 1))
                    nc.vector.tensor_copy(out=ctxT[:, c_tout, :], in_=ps)
                ctx_ops.append(ctx_mm)
            # space the rowsum ops out so their DVE/ACT steps hide behind MMs
            ops.append(ctx_ops[0])
            ops.append(ctx_ops[1])
            ops.append(rs_recip)
            ops.append(ctx_ops[2])
            ops.append(ctx_ops[3])

            for qs in range(DC):
                def out_mm(qs=qs):
                    ps = ps_work.tile([P, D], f32, tag="work", name=f"op_{ib}_{qs}")
                    for c_tout in range(DC):
                        nc.tensor.matmul(ps,
                                         ctxT[:, c_tout, qs * P:(qs + 1) * P],
                                         wo_b[:, c_tout, :],
                                         start=(c_tout == 0), stop=(c_tout == DC - 1))
                    ot_s = outp.tile([P, D], f32, tag="out_s", name=f"ots_{ib}_{qs}")
                    nc.scalar.activation(ot_s, ps, AF.Copy,
                                         scale=rc_sb[:, qs:qs + 1])
                    ot = outp.tile([P, D], f32, tag="out_t", name=f"ot_{ib}_{qs}")
                    nc.vector.tensor_tensor(ot, ot_s, bias_rep, OP.add)
                    nc.sync.dma_start(
                        out_dram[ib * IB + qs * P: ib * IB + (qs + 1) * P, :], ot)
                ops.append(out_mm)
            return ops

        # ---- target chunk: cast + transpose into tgtT (rides block 0) ----
        def tgt_op(scc):
            nc.vector.tensor_copy(out=tgt_h[:, scc, :], in_=tstage[scc])
            pst = ps_work.tile([P, D], f32, tag="work", name=f"pst_t{scc}")
            for dc in range(DC):
                transpose16(pst[:, dc * P:(dc + 1) * P],
                            tgt_h[:, scc, dc * P:(dc + 1) * P])
            nc.scalar.activation(
                tgtT.rearrange("p c (s q) -> p c s q", s=SC)[:, :, scc, :],
                pst.rearrange("p (c q) -> p c q", c=DC),
                AF.Copy)

        # ---- block 0 query side (standalone: nothing else to overlap) ----
        ops0 = qside_ops(0)
        for op in ops0[:4]:      # chunk casts + transposes
            op()
        for dc in range(DC):
            nc.vector.tensor_copy(out=wq_h[:, dc, :], in_=wq_f[:, dc, :])
        for op in ops0[4:8]:     # q projection
            op()
        # wkT: PE-transpose wk into [tout, tin] layout (needs wk_h cast)
        for dc in range(DC):
            nc.vector.tensor_copy(out=wk_h[:, dc, :], in_=wk_f[:, dc, :])
        for c_tin in range(DC):
            pst = ps_work.tile([P, D], f32, tag="work", name=f"wkt_{c_tin}")
            for c_tout in range(DC):
                transpose16(pst[:, c_tout * P:(c_tout + 1) * P],
                            wk_h[:, c_tin, c_tout * P:(c_tout + 1) * P])
            nc.scalar.activation(
                wkT_h.rearrange("p c (ci q) -> p c ci q", ci=DC)[:, :, c_tin, :],
                pst.rearrange("p (c q) -> p c q", c=DC),
                AF.Copy)
        for op in ops0[8:]:      # A
            op()

        # ---- attention blocks ----
        tails = None
        for ib in range(NB):
            U_ps = ps_u.tile([P, DC * IB], f32, tag="U", name=f"U_{ib}")
            acc = smallp.tile([P, IB], f32, tag="rs_acc", name=f"rsacc_{ib}")
            U_sb = up.tile([P, DC, IB], bf16, tag="U", name=f"Usb_{ib}")
            pte_tiles = [None] * SC

            def scores(scc, ib=ib, acc=acc, pte_tiles=pte_tiles):
                pt_ps = ps_work.tile([P, IB], f32, tag="work", name=f"pt_{ib}_{scc}")
                for c_tin in range(DC):
                    nc.tensor.matmul(pt_ps,
                                     tgtT[:, c_tin, scc * P:(scc + 1) * P],
                                     A_sb[:, c_tin, ib * IB:(ib + 1) * IB],
                                     start=(c_tin == 0), stop=(c_tin == DC - 1))
                pte = ptp.tile([P, IB], f16, tag="pte", name=f"pte_{ib}_{scc}")
                nc.scalar.activation(pte, pt_ps, AF.Exp, bias=negc_col)
                pte_tiles[scc] = pte
                if scc == 0:
                    nc.vector.tensor_copy(out=acc, in_=pte)
                else:
                    nc.vector.tensor_tensor(acc, acc, pte, OP.add)

            def u_mm(scc, ib=ib, U_ps=U_ps, pte_tiles=pte_tiles):
                pte = pte_tiles[scc]
                for c_tin in range(DC):
                    nc.tensor.matmul(U_ps[:, c_tin * IB:(c_tin + 1) * IB],
                                     tgt_h[:, scc, c_tin * P:(c_tin + 1) * P],
                                     pte,
                                     start=(scc == 0), stop=(scc == SC - 1))

            pending = []
            if ib == 0:
                pending += qside_ops(1)
                pending += wcast_ops()
                pending += [bias_setup_op]
            else:
                pending += tails
                if ib + 1 < NB:
                    pending += qside_ops(ib + 1)
            tail_iter = iter(pending)

            if ib == 0:
                tgt_op(0)
            for scc in range(SC):
                if ib == 0 and scc + 1 < SC:
                    tgt_op(scc + 1)
                scores(scc)
                if scc >= 1:
                    u_mm(scc - 1)
                if scc >= 2:
                    op = next(tail_iter, None)
                    if op is not None:
                        op()
            for op in tail_iter:
                op()
            u_mm(SC - 1)
            # evacuate U per chunk so the next block's U matmuls can begin
            for c_tin in range(DC):
                nc.scalar.activation(U_sb[:, c_tin, :],
                                     U_ps[:, c_tin * IB:(c_tin + 1) * IB], AF.Copy)
            tails = make_tail(ib, U_sb, acc)

        for op in tails:
            op()

    nc.compile()
    return nc


def _get_nc():
    if "nc" not in _CACHED:
        _CACHED["nc"] = _build_program()
    return _CACHED["nc"]


def _make_in_maps(query, target, wq, bq, wk, bk, wv, bv, wo, bo):
    query = np.asarray(query, dtype=np.float32)
    target = np.asarray(target, dtype=np.float32)
    consts = {
        "wq": np.asarray(wq, np.float32), "bq": np.asarray(bq, np.float32),
        "wk": np.asarray(wk, np.float32), "bk": np.asarray(bk, np.float32),
        "wv": np.asarray(wv, np.float32), "bv": np.asarray(bv, np.float32),
        "wo": np.asarray(wo, np.float32), "bo": np.asarray(bo, np.float32),
        "ident16": np.eye(128, dtype=np.float16),
        "identf": np.eye(128, dtype=np.float32),
    }
    in_maps = []
    for core in range(8):
        b, h = divmod(core, 2)
        in_maps.append({
            "query": np.ascontiguousarray(query[b, h * LQH:(h + 1) * LQH]),
            # faithful to the torch reshape: raw reinterpret of [512, 4096]
            "target": np.ascontiguousarray(target[b]).reshape(S, D),
            **consts,
        })
    return in_maps


def kernel(query, target, wq, bq, wk, bk, wv, bv, wo, bo):
    from concourse import bass_utils
    nc = _get_nc()
    in_maps = _make_in_maps(query, target, wq, bq, wk, bk, wv, bv, wo, bo)
    res = bass_utils.run_bass_kernel_spmd(nc, in_maps, core_ids=list(range(8)))
    out = np.empty((B, LQ, D), np.float32)
    for core in range(8):
        b, h = divmod(core, 2)
        out[b, h * LQH:(h + 1) * LQH] = res.results[core]["out"]
    return out


# revision 34
# speedup vs baseline: 1.0164x; 1.0164x over previous
"""Trainium2 Bass kernel for nn_CrossAttention (B=4, LQ=4096, S=4096, D=512).

Sharding: data-parallel over (batch, query-half): core = 2*b + half.
Each core handles one batch element and 2048 query rows.

Algebraic fusion removes the per-pair duplicated K/V projections:
  scores^T = tgt @ A          with A = wk @ q^T   (bk cancels in softmax)
  ctx^T    = wv^T @ U         with U = tgt^T @ probs^T
so the only S-sized matmuls are the two intrinsic attention GEMMs; all
projection-sized work scales with the per-core query count.

Precision: scores path runs in fp16 (3 extra mantissa bits over bf16 at the
same PE rate). Softmax skips max-subtraction but shifts by a global C=18 so
the unnormalized exp fits fp16; the shift cancels exactly in the final
per-row normalization. U / ctx / output projection run bf16.

Scheduling: everything outside the two big attention GEMMs is expressed as
small closures injected one-per-step into the attention loops so the PE
never drains: the target load/cast/transpose stream rides block 0's loop
(staggered one chunk ahead so the ACT copy is hidden), each block's query
projections ride the previous block's loop, and each block's tail (rowsum
reciprocal, ctx and output projections) rides the next block's loop.
Engine queues are ordered by data-arrival time so no in-order queue blocks
on a late DMA.
"""

import numpy as np

B, LQ, S = 4, 4096, 4096
D = 512          # SRC == TGT == 512
P = 128
LQH = LQ // 2    # 2048 query rows per core
DC = D // P      # 4 chunks of the feature dims
SC = S // P      # 32 s-chunks of 128
IB = 512         # query block width
NB = LQH // IB   # 4 query blocks
CEXP = 18.0      # global softmax shift: exp(score - CEXP) fits fp16

_CACHED = {}


def _build_program():
    import concourse.bass as bass
    import concourse.mybir as mybir
    import concourse.tile as tile
    from concourse import bacc
    from contextlib import ExitStack

    f32 = mybir.dt.float32
    bf16 = mybir.dt.bfloat16
    f16 = mybir.dt.float16
    AF = mybir.ActivationFunctionType
    OP = mybir.AluOpType

    nc = bacc.Bacc("TRN2", target_bir_lowering=False, debug=False, num_devices=8)

    query = nc.dram_tensor("query", [LQH, D], f32, kind="ExternalInput").ap()
    target = nc.dram_tensor("target", [S, D], f32, kind="ExternalInput").ap()
    w_dram = {}
    b_dram = {}
    for nm in ("wq", "wk", "wv", "wo"):
        w_dram[nm] = nc.dram_tensor(nm, [D, D], f32, kind="ExternalInput").ap()
    for nm in ("bq", "bk", "bv", "bo"):
        b_dram[nm] = nc.dram_tensor(nm, [D], f32, kind="ExternalInput").ap()
    ident16_d = nc.dram_tensor("ident16", [P, P], f16, kind="ExternalInput").ap()
    identf_d = nc.dram_tensor("identf", [P, P], f32, kind="ExternalInput").ap()
    bias_scr = nc.dram_tensor("bias_scr", [D], f32, kind="Internal").ap()
    out_dram = nc.dram_tensor("out", [LQH, D], f32, kind="ExternalOutput").ap()

    with tile.TileContext(nc) as tc, ExitStack() as ctx:
        const = ctx.enter_context(tc.tile_pool(name="const", bufs=1))
        big = ctx.enter_context(tc.tile_pool(name="big", bufs=1))
        wstage = ctx.enter_context(tc.tile_pool(name="wstage", bufs=2))
        ld = ctx.enter_context(tc.tile_pool(name="ld", bufs=4))
        tcp = ctx.enter_context(tc.tile_pool(name="tcp", bufs=10))
        qp = ctx.enter_context(tc.tile_pool(name="qp", bufs=2))
        ptp = ctx.enter_context(tc.tile_pool(name="ptp", bufs=4))
        up = ctx.enter_context(tc.tile_pool(name="up", bufs=2))
        outp = ctx.enter_context(tc.tile_pool(name="outp", bufs=2))
        smallp = ctx.enter_context(tc.tile_pool(name="smallp", bufs=2))
        # PSUM budget: work 3 banks + U 4 banks + rs1 1 bank = 8
        ps_work = ctx.enter_context(tc.tile_pool(name="ps_work", bufs=3, space="PSUM"))
        ps_u = ctx.enter_context(tc.tile_pool(name="ps_u", bufs=1, space="PSUM"))
        ps_rs1 = ctx.enter_context(tc.tile_pool(name="ps_rs1", bufs=1, space="PSUM"))

        # ---- constants ----
        ident16 = const.tile([P, P], f16, tag="ident16", name="ident16")
        nc.sync.dma_start(ident16, ident16_d)

        def transpose16(out_f32, in_f16):
            # fp16 128x128 transpose as a regular matmul: out = in^T @ I
            nc.tensor.matmul(out_f32, in_f16, ident16, start=True, stop=True)

        ones_col = const.tile([P, 1], f32, tag="ones_col", name="ones_col")
        nc.vector.memset(ones_col, 1.0)
        negc_col = const.tile([P, 1], f32, tag="negc_col", name="negc_col")
        nc.vector.memset(negc_col, -CEXP)
        rstage = const.tile([P, IB], f32, tag="rstage", name="rstage")
        nc.vector.memset(rstage, 0.0)

        bq_col = const.tile([P, DC], f32, tag="bq_col", name="bq_col")
        bv_col = const.tile([P, DC], f32, tag="bv_col", name="bv_col")
        bo_row = const.tile([1, D], f32, tag="bo_row", name="bo_row")

        # ---- persistent SBUF arrays ----
        wq_h = const.tile([P, DC, D], f16, tag="wq_h", name="wq_h")
        wk_h = const.tile([P, DC, D], f16, tag="wk_h", name="wk_h")
        wkT_h = const.tile([P, DC, D], f16, tag="wkT_h", name="wkT_h")
        wv_b = const.tile([P, DC, D], bf16, tag="wv_b", name="wv_b")
        wo_b = const.tile([P, DC, D], bf16, tag="wo_b", name="wo_b")
        A_sb = big.tile([P, DC, LQH], f16, tag="A_sb", name="A_sb")
        tgtT = big.tile([P, DC, S], f16, tag="tgtT", name="tgtT")
        tgt_h = big.tile([P, SC, D], f16, tag="tgt_h", name="tgt_h")
        bias_rep = const.tile([P, D], f32, tag="bias_rep", name="bias_rep")

        # ---- DMA issue: ONE queue, strict arrival-priority order ----
        # (transfers from different queues merge on the rings, so priority
        # only holds within a single queue)
        qstage = {}
        wq_f = wstage.tile([P, DC, D], f32, tag="w_stage", name="wq_f32")
        wk_f = wstage.tile([P, DC, D], f32, tag="w_stage", name="wk_f32")
        for cc in range(IB // P):
            chunk = ld.tile([P, D], f32, tag="qc", name=f"qc_0_{cc}")
            nc.sync.dma_start(chunk, query[cc * P:(cc + 1) * P, :])
            qstage[(0, cc)] = chunk
            nc.sync.dma_start(wq_f[:, cc, :], w_dram["wq"][cc * P:(cc + 1) * P, :])
        nc.sync.dma_start(out=bq_col, in_=b_dram["bq"].rearrange("(c p) -> p c", p=P))
        for dc in range(DC):
            nc.sync.dma_start(wk_f[:, dc, :], w_dram["wk"][dc * P:(dc + 1) * P, :])
        tstage = []
        for scc in range(8):
            chunk = tcp.tile([P, D], f32, tag="tc", name=f"tc_{scc}")
            nc.sync.dma_start(chunk, target[scc * P:(scc + 1) * P, :])
            tstage.append(chunk)
        for ibk in range(1, NB):
            for cc in range(IB // P):
                chunk = ld.tile([P, D], f32, tag="qc", name=f"qc_{ibk}_{cc}")
                nc.sync.dma_start(chunk, query[ibk * IB + cc * P: ibk * IB + (cc + 1) * P, :])
                qstage[(ibk, cc)] = chunk
        nc.sync.dma_start(out=bv_col, in_=b_dram["bv"].rearrange("(c p) -> p c", p=P))
        nc.sync.dma_start(out=bo_row, in_=b_dram["bo"])
        wv_f = wstage.tile([P, DC, D], f32, tag="w_stage", name="wv_f32")
        wo_f = wstage.tile([P, DC, D], f32, tag="w_stage", name="wo_f32")
        for dc in range(DC):
            nc.sync.dma_start(wv_f[:, dc, :], w_dram["wv"][dc * P:(dc + 1) * P, :])
        for dc in range(DC):
            nc.sync.dma_start(wo_f[:, dc, :], w_dram["wo"][dc * P:(dc + 1) * P, :])
        identf = const.tile([P, P], f32, tag="identf", name="identf")
        nc.sync.dma_start(identf, identf_d)
        for scc in range(8, SC):
            chunk = tcp.tile([P, D], f32, tag="tc", name=f"tc_{scc}")
            nc.sync.dma_start(chunk, target[scc * P:(scc + 1) * P, :])
            tstage.append(chunk)

        # ---- query-side compute pieces for one ib block ----
        def qside_ops(ibk):
            """Closures: 4 transpose-chunk ops, 4 q-proj ops, 4 A ops."""
            qinT = qp.tile([P, DC, IB], f16, tag="qinT", name=f"qinT{ibk}")
            qT = qp.tile([P, DC, IB], f16, tag="qT", name=f"qT{ibk}")
            ops = []

            for cc in range(IB // P):
                def chunk_op(cc=cc):
                    chunk = qstage[(ibk, cc)]
                    cast = ld.tile([P, D], f16, tag="qch", name=f"qch_{ibk}_{cc}")
                    nc.vector.tensor_copy(out=cast, in_=chunk)
                    psv = ps_work.tile([P, D], f32, tag="work", name=f"pst_q{ibk}_{cc}")
                    for dc in range(DC):
                        transpose16(psv[:, dc * P:(dc + 1) * P],
                                    cast[:, dc * P:(dc + 1) * P])
                    c = cc * P
                    nc.scalar.activation(qinT[:, :, c:c + P],
                                         psv.rearrange("p (c q) -> p c q", c=DC),
                                         AF.Copy)
                ops.append(chunk_op)

            for tcc in range(DC):
                def qproj_op(tcc=tcc):
                    ps = ps_work.tile([P, IB], f32, tag="work", name=f"psq_{tcc}_{ibk}")
                    for dc in range(DC):
                        nc.tensor.matmul(ps, wq_h[:, dc, tcc * P:(tcc + 1) * P],
                                         qinT[:, dc, :],
                                         start=(dc == 0), stop=(dc == DC - 1))
                    nc.vector.tensor_tensor(qT[:, tcc, :], ps,
                                            bq_col[:, tcc:tcc + 1].to_broadcast([P, IB]),
                                            OP.add)
                ops.append(qproj_op)

            for c_tin in range(DC):
                def a_op(c_tin=c_tin):
                    ps = ps_work.tile([P, IB], f32, tag="work", name=f"psA_{c_tin}_{ibk}")
                    for c_tout in range(DC):
                        nc.tensor.matmul(
                            ps,
                            wkT_h[:, c_tout, c_tin * P:(c_tin + 1) * P],
                            qT[:, c_tout, :],
                            start=(c_tout == 0), stop=(c_tout == DC - 1))
                    nc.scalar.activation(A_sb[:, c_tin, ibk * IB:(ibk + 1) * IB],
                                         ps, AF.Copy)
                ops.append(a_op)
            return ops

        # ---- deferred weight casts + output-bias setup (ride block 0) ----
        def wcast_ops():
            ops = []
            for dst, src in ((wv_b, wv_f), (wo_b, wo_f)):
                for dc in range(DC):
                    def cast_op(dst=dst, src=src, dc=dc):
                        nc.vector.tensor_copy(out=dst[:, dc, :], in_=src[:, dc, :])
                    ops.append(cast_op)
            return ops

        def bias_setup_op():
            bv_colb = const.tile([P, DC], bf16, tag="bv_colb", name="bv_colb")
            nc.vector.tensor_copy(out=bv_colb, in_=bv_col)
            bvwo_ps = ps_rs1.tile([1, D], f32, tag="rs1", name="bvwo")
            for c in range(DC):
                nc.tensor.matmul(bvwo_ps, bv_colb[:, c:c + 1], wo_b[:, c, :],
                                 start=(c == 0), stop=(c == DC - 1))
            bias_tot = const.tile([1, D], f32, tag="bias_tot", name="bias_tot")
            nc.vector.tensor_tensor(bias_tot, bvwo_ps, bo_row, OP.add)
            nc.sync.dma_start(out=bias_scr, in_=bias_tot)
            bcast = bass.AP(tensor=bias_scr.tensor, offset=bias_scr.offset,
                            ap=[[0, P]] + list(bias_scr.ap))
            nc.sync.dma_start(out=bias_rep, in_=bcast)

        # ---- per-block tail: rowsum recip, ctx projection, out projection ----
        def make_tail(ib, U_sb, acc):
            ops = []
            rc_sb = smallp.tile([P, DC], f32, tag="rc_sb", name=f"rc_{ib}")
            ctxT = outp.tile([P, DC, IB], bf16, tag="ctx", name=f"ctxT_{ib}")

            def rs_collapse():
                rs_ps = ps_rs1.tile([1, IB], f32, tag="rs1", name=f"rs_{ib}")
                nc.tensor.matmul(rs_ps, ones_col, acc, start=True, stop=True)
                nc.vector.tensor_copy(out=rstage[0:1, :], in_=rs_ps)
            ops.append(rs_collapse)

            def rs_recip():
                rt_ps = ps_work.tile([P, IB], f32, tag="work", name=f"rt_{ib}")
                for ic in range(DC):
                    nc.tensor.transpose(rt_ps[:, ic * P:(ic + 1) * P],
                                        rstage[:, ic * P:(ic + 1) * P], identf)
                rsum_col = smallp.tile([P, DC], f32, tag="rsum_col", name=f"rsc_{ib}")
                nc.scalar.activation(rsum_col,
                                     rt_ps.rearrange("p (c q) -> p c q", c=DC)[:, :, 0],
                                     AF.Copy)
                nc.vector.reciprocal(rc_sb, rsum_col)

            ctx_ops = []
            for c_tout in range(DC):
                def ctx_mm(c_tout=c_tout):
                    ps = ps_work.tile([P, IB], f32, tag="work", name=f"ctx_{ib}_{c_tout}")
                    for c_tin in range(DC):
                        nc.tensor.matmul(ps,
                                         wv_b[:, c_tin, c_tout * P:(c_tout + 1) * P],
                                         U_sb[:, c_tin, :],
                                         start=(c_tin == 0), stop=(c_tin == DC - 1))
                    nc.vector.tensor_copy(out=ctxT[:, c_tout, :], in_=ps)
                ctx_ops.append(ctx_mm)
            # space the rowsum ops out so their DVE/ACT steps hide behind MMs
            ops.append(ctx_ops[0])
            ops.append(ctx_ops[1])
            ops.append(rs_recip)
            ops.append(ctx_ops[2])
            ops.append(ctx_ops[3])

            for qs in range(DC):
                def out_mm(qs=qs):
                    ps = ps_work.tile([P, D], f32, tag="work", name=f"op_{ib}_{qs}")
                    for c_tout in range(DC):
                        nc.tensor.matmul(ps,
                                         ctxT[:, c_tout, qs * P:(qs + 1) * P],
                                         wo_b[:, c_tout, :],
                                         start=(c_tout == 0), stop=(c_tout == DC - 1))
                    ot_s = outp.tile([P, D], f32, tag="out_s", name=f"ots_{ib}_{qs}")
                    nc.scalar.activation(ot_s, ps, AF.Copy,
                                         scale=rc_sb[:, qs:qs + 1])
                    ot = outp.tile([P, D], f32, tag="out_t", name=f"ot_{ib}_{qs}")
                    nc.vector.tensor_tensor(ot, ot_s, bias_rep, OP.add)
                    nc.sync.dma_start(
                        out_dram[ib * IB + qs * P: ib * IB + (qs + 1) * P, :], ot)
                ops.append(out_mm)
            return ops

        # ---- target chunk: cast + transpose into tgtT (rides block 0) ----
        def tgt_op(scc):
            nc.vector.tensor_copy(out=tgt_h[:, scc, :], in_=tstage[scc])
            pst = ps_work.tile([P, D], f32, tag="work", name=f"pst_t{scc}")
            for dc in range(DC):
                transpose16(pst[:, dc * P:(dc + 1) * P],
                            tgt_h[:, scc, dc * P:(dc + 1) * P])
            nc.scalar.activation(
                tgtT.rearrange("p c (s q) -> p c s q", s=SC)[:, :, scc, :],
                pst.rearrange("p (c q) -> p c q", c=DC),
                AF.Copy)

        # ---- block 0 query side (standalone: nothing else to overlap) ----
        ops0 = qside_ops(0)
        for op in ops0[:4]:      # chunk casts + transposes
            op()
        for dc in range(DC):
            nc.vector.tensor_copy(out=wq_h[:, dc, :], in_=wq_f[:, dc, :])
        for op in ops0[4:8]:     # q projection
            op()
        # wkT: PE-transpose wk into [tout, tin] layout (needs wk_h cast)
        for dc in range(DC):
            nc.vector.tensor_copy(out=wk_h[:, dc, :], in_=wk_f[:, dc, :])
        for c_tin in range(DC):
            pst = ps_work.tile([P, D], f32, tag="work", name=f"wkt_{c_tin}")
            for c_tout in range(DC):
                transpose16(pst[:, c_tout * P:(c_tout + 1) * P],
                            wk_h[:, c_tin, c_tout * P:(c_tout + 1) * P])
            nc.scalar.activation(
                wkT_h.rearrange("p c (ci q) -> p c ci q", ci=DC)[:, :, c_tin, :],
                pst.rearrange("p (c q) -> p c q", c=DC),
                AF.Copy)
        for op in ops0[8:]:      # A
            op()

        # ---- attention blocks ----
        tails = None
        for ib in range(NB):
            U_ps = ps_u.tile([P, DC * IB], f32, tag="U", name=f"U_{ib}")
            acc = smallp.tile([P, IB], f32, tag="rs_acc", name=f"rsacc_{ib}")
            U_sb = up.tile([P, DC, IB], bf16, tag="U", name=f"Usb_{ib}")
            pte_tiles = [None] * SC

            def scores(scc, ib=ib, acc=acc, pte_tiles=pte_tiles):
                pt_ps = ps_work.tile([P, IB], f32, tag="work", name=f"pt_{ib}_{scc}")
                for c_tin in range(DC):
                    nc.tensor.matmul(pt_ps,
                                     tgtT[:, c_tin, scc * P:(scc + 1) * P],
                                     A_sb[:, c_tin, ib * IB:(ib + 1) * IB],
                                     start=(c_tin == 0), stop=(c_tin == DC - 1))
                pte = ptp.tile([P, IB], f16, tag="pte", name=f"pte_{ib}_{scc}")
                nc.scalar.activation(pte, pt_ps, AF.Exp, bias=negc_col)
                pte_tiles[scc] = pte
                if scc == 0:
                    nc.vector.tensor_copy(out=acc, in_=pte)
                else:
                    nc.vector.tensor_tensor(acc, acc, pte, OP.add)

            def u_mm(scc, ib=ib, U_ps=U_ps, pte_tiles=pte_tiles):
                pte = pte_tiles[scc]
                for c_tin in range(DC):
                    nc.tensor.matmul(U_ps[:, c_tin * IB:(c_tin + 1) * IB],
                                     tgt_h[:, scc, c_tin * P:(c_tin + 1) * P],
                                     pte,
                                     start=(scc == 0), stop=(scc == SC - 1))

            pending = []
            if ib == 0:
                pending += qside_ops(1)
                pending += wcast_ops()
                pending += [bias_setup_op]
            else:
                pending += tails
                if ib + 1 < NB:
                    pending += qside_ops(ib + 1)
            tail_iter = iter(pending)

            if ib == 0:
                tgt_op(0)
            for scc in range(SC):
                if ib == 0 and scc + 1 < SC:
                    tgt_op(scc + 1)
                scores(scc)
                if scc >= 1:
                    u_mm(scc - 1)
                # block 0's injected ops wait for later-arriving DMA data
                if scc >= (6 if ib == 0 else 2):
                    op = next(tail_iter, None)
                    if op is not None:
                        op()
            for op in tail_iter:
                op()
            u_mm(SC - 1)
            # evacuate U per chunk so the next block's U matmuls can begin
            for c_tin in range(DC):
                nc.scalar.activation(U_sb[:, c_tin, :],
                                     U_ps[:, c_tin * IB:(c_tin + 1) * IB], AF.Copy)
            tails = make_tail(ib, U_sb, acc)

        for op in tails:
            op()

    nc.compile()
    return nc


def _get_nc():
    if "nc" not in _CACHED:
        _CACHED["nc"] = _build_program()
    return _CACHED["nc"]


def _make_in_maps(query, target, wq, bq, wk, bk, wv, bv, wo, bo):
    query = np.asarray(query, dtype=np.float32)
    target = np.asarray(target, dtype=np.float32)
    consts = {
        "wq": np.asarray(wq, np.float32), "bq": np.asarray(bq, np.float32),
        "wk": np.asarray(wk, np.float32), "bk": np.asarray(bk, np.float32),
        "wv": np.asarray(wv, np.float32), "bv": np.asarray(bv, np.float32),
        "wo": np.asarray(wo, np.float32), "bo": np.asarray(bo, np.float32),
        "ident16": np.eye(128, dtype=np.float16),
        "identf": np.eye(128, dtype=np.float32),
    }
    in_maps = []
    for core in range(8):
        b, h = divmod(core, 2)
        in_maps.append({
            "query": np.ascontiguousarray(query[b, h * LQH:(h + 1) * LQH]),
            # faithful to the torch reshape: raw reinterpret of [512, 4096]
            "target": np.ascontiguousarray(target[b]).reshape(S, D),
            **consts,
        })
    return in_maps


def kernel(query, target, wq, bq, wk, bk, wv, bv, wo, bo):
    from concourse import bass_utils
    nc = _get_nc()
    in_maps = _make_in_maps(query, target, wq, bq, wk, bk, wv, bv, wo, bo)
    res = bass_utils.run_bass_kernel_spmd(nc, in_maps, core_ids=list(range(8)))
    out = np.empty((B, LQ, D), np.float32)
    for core in range(8):
        b, h = divmod(core, 2)
        out[b, h * LQH:(h + 1) * LQH] = res.results[core]["out"]
    return out
